# revision 73
# baseline (speedup 1.0000x reference)
"""Tensor-parallel LlamaAttention (GQA + RoPE + causal) for 8 trn2 NeuronCores.

Sharding: column-parallel q/k/v by head (4 q-heads, 1 kv-head per core),
attention computed locally per head, then ROW-parallel o_proj on the local
head slice producing a partial output y_part[BT, H]; the host sums the 8
partials (the "all-reduce" of the RowParallel structure is done at gather
time on the host, so no on-device collective is needed).

Layout strategy (token dim = free dim, everything transposed):
  xr[p, c, t]  host-pre-transposed x (= x[t, c*128+p], bf16); weights are
               likewise host-flattened to [128, ...] so each matrix loads
               in a handful of big DMAs
  qT/kT[d, t]  from projection matmuls (lhsT=W block, rhs=x block)
  v[t, d]      natural layout (lhsT=x block, rhs=Wv block)
  S^T[k, q]    = matmul(lhsT=kT slice, rhs=qT slice)
  P^T          = exp(scale*S^T) via ACT (scores bounded ~|8|, exp safe)
  O^T[d, q]   += matmul(lhsT=v tile, rhs=P^T)    (PSUM accum over k)
  L (denoms)   = two bf16 DVE accumulators of P tiles, then a tiny
                 ones-matmul on PE broadcasts the partition-sum into PSUM
  attnT        = O^T * reciprocal_approx_fast(L) (SBUF, bf16)
  y_part      += matmul(lhsT=attnT slice, rhs=wo rows) -> DRAM, host-summed

Pipelining: projection chunk i (512 tokens), attention chunk i, and o_proj
micro-ops for chunk i-1 are interleaved in the instruction stream, so the
PE always has independent work during DMA warmup and exp-latency stalls.
"""

import math
import sys

import numpy as np

sys.path.insert(0, "/opt/trn_rl_repo")

import ml_dtypes  # noqa: E402

from concourse import bacc, mybir, tile  # noqa: E402
from concourse.bass_utils import run_bass_kernel_spmd  # noqa: E402

F32 = mybir.dt.float32
BF16 = mybir.dt.bfloat16
NCORES = 8
P = 128  # partitions / head dim
QB = 512  # q-block (PSUM free dim)
KB = 128  # k-block (contraction tile)

_CACHE = {}


def build_program(B, S, H, NH, NKV):
    nc = bacc.Bacc("TRN2", num_devices=NCORES)

    BT = B * S  # total tokens (4096)
    NHC = NH // NCORES  # q heads per core (4)
    DQ = NHC * P  # per-core q width (512)
    HB = H // P  # h blocks (32)
    TB = BT // QB  # token super-blocks (8)
    QBB = S // QB  # q blocks per batch (4)
    RB = QB // KB  # diag tiles per q block (4)

    # weights host-prearranged to [128 partitions, flat] so each matrix
    # loads with a handful of big DMAs (dma_start issue costs ~565ns each
    # on the sync sequencer; per-tile loads serialize the warmup)
    # x host-prearranged to [128, HB, BT]: xr[p, c, t] = x[t, c*128 + p]
    xr = nc.declare_dram_parameter("xr", [P, HB, BT], BF16, isOutput=False)
    wq_r = nc.declare_dram_parameter("wq_r", [P, HB * DQ], BF16, isOutput=False)
    wk_r = nc.declare_dram_parameter("wk_r", [P, HB * P], BF16, isOutput=False)
    wv_r = nc.declare_dram_parameter("wv_r", [P, HB * P], BF16, isOutput=False)
    wo_rr = nc.declare_dram_parameter("wo_rr", [P, NHC * H], BF16, isOutput=False)
    cos_t = nc.declare_dram_parameter("cos_t", [P, BT], F32, isOutput=False)
    sinx_t = nc.declare_dram_parameter("sinx_t", [P, BT], F32, isOutput=False)
    consts_t = nc.declare_dram_parameter("consts_t", [2, P, P], BF16, isOutput=False)
    y_c = nc.declare_dram_parameter("y_c", [BT, H], BF16, isOutput=True)

    scale = 1.0 / math.sqrt(P)

    with tile.TileContext(nc) as tc:
        with (
            tc.tile_pool(name="const", bufs=1) as constp,
            tc.tile_pool(name="persist", bufs=1) as persist,
            tc.tile_pool(name="qt", bufs=2 * NHC + 1) as qt_p,
            tc.tile_pool(name="xin", bufs=6) as xin_p,
            tc.tile_pool(name="wqkv", bufs=1) as w_p,
            tc.tile_pool(name="tabs", bufs=1) as tab_p,
            tc.tile_pool(name="ropetmp", bufs=1) as rt_p,
            tc.tile_pool(name="pP", bufs=6) as p_p,
            tc.tile_pool(name="lacc", bufs=2) as lacc_p,
            tc.tile_pool(name="rt2", bufs=3) as rt2_pool,
            tc.tile_pool(name="aout", bufs=3 * NHC + 2) as ao_p,
            tc.tile_pool(name="yout", bufs=2) as y_p,
            tc.tile_pool(name="psQS", bufs=2, space="PSUM") as qs_p,
            tc.tile_pool(name="psO", bufs=2, space="PSUM") as pso_p,
            tc.tile_pool(name="psV", bufs=2, space="PSUM") as psv_p,
            tc.tile_pool(name="psY", bufs=2, space="PSUM") as psy_p,
        ):
            # ones matrix for the L (softmax denominator) row-sum matmuls
            ones_sb = constp.tile([P, P], BF16, tag="ones")
            # triangular causal mask for the diagonal subtile: tri[k,q]=k<=q
            tri_sb = constp.tile([P, P], BF16, tag="tri")

            # persistent per-core activations (bf16)
            kT = persist.tile([P, BT], BF16, tag="kT")
            vt = [
                persist.tile([P, P], BF16, tag=f"v{i}", name=f"v{i}")
                for i in range(BT // P)
            ]
            # o_proj weights (rows for this core's heads), resident whole run
            # wo_all[g][:, (h%2)*H + j] = wo[head 2g+h row, col j]
            NWOG = (NHC + 1) // 2  # wo DMA groups of up to 2 heads
            wo_all = [
                persist.tile(
                    [P, min(2, NHC - 2 * g) * H], BF16, tag=f"wo{g}", name=f"wo{g}"
                )
                for g in range(NWOG)
            ]

            def wo_sl(h, c0, c1):
                return wo_all[h // 2][:, (h % 2) * H + c0 : (h % 2) * H + c1]

            # q/k/v projection weights, resident whole run, flat layout:
            # group g covers hb [s, s+l): tile[:, (hb-s)*DQ + j] = wq[hb*128+p, j]
            # (first group small so the very first q chain starts ~immediately)
            wq_groups = [(0, min(2, HB))]
            while sum(l for _, l in wq_groups) < HB:
                s = sum(l for _, l in wq_groups)
                wq_groups.append((s, min(6 if s == 2 else 8, HB - s)))
            wq_g = [
                w_p.tile([P, l * DQ], BF16, tag=f"wq{gi}", name=f"wq{gi}")
                for gi, (s, l) in enumerate(wq_groups)
            ]
            wk_all = w_p.tile([P, HB * P], BF16, tag="wk_all")
            wv_all = w_p.tile([P, HB * P], BF16, tag="wv_all")

            def wq_sl(hb, c0, c1):
                for gi, (s, l) in enumerate(wq_groups):
                    if s <= hb < s + l:
                        return wq_g[gi][:, (hb - s) * DQ + c0 : (hb - s) * DQ + c1]

            microq = []  # pending o_proj micro-ops

            def drain(n):
                for _ in range(min(n, len(microq))):
                    microq.pop(0)()

            def rope(dst, ps, cos_sb, sinx_sb):
                """dst = ps*cos + shift64(ps)*sinx (all [128,QB])"""
                t1 = rt_p.tile([P, QB], F32, tag="ropet1")
                t2 = rt_p.tile([P, QB], F32, tag="ropet2")
                nc.vector.tensor_tensor(t1, ps, cos_sb, mybir.AluOpType.mult)
                hh = P // 2
                nc.vector.tensor_tensor(
                    t2[0:hh], ps[hh:P], sinx_sb[0:hh], mybir.AluOpType.mult
                )
                nc.vector.tensor_tensor(
                    t2[hh:P], ps[0:hh], sinx_sb[hh:P], mybir.AluOpType.mult
                )
                nc.vector.tensor_tensor(dst, t1, t2, mybir.AluOpType.add)

            def emit_proj(tb):
                """Projections + rope for token block tb -> per-head q tiles."""
                t0 = tb * QB
                cos_sb = tab_p.tile([P, QB], F32, tag="cos")
                sinx_sb = tab_p.tile([P, QB], F32, tag="sinx")
                nc.sync.dma_start(out=cos_sb, in_=cos_t[:, t0 : t0 + QB])
                nc.sync.dma_start(out=sinx_sb, in_=sinx_t[:, t0 : t0 + QB])

                # stream x tiles; on tb 0 the x groups mirror the staged wq
                # groups (first group tiny) and interleave the weight loads,
                # so the first q chain starts within a few us
                xts = []
                XG = (HB + 3) // 4  # hb blocks per x DMA group (steady state)
                if tb == 0:
                    xgroups = list(wq_groups)
                else:
                    xgroups = [
                        (g * XG, min((g + 1) * XG, HB) - g * XG)
                        for g in range(4)
                        if g * XG < HB
                    ]
                for gx, (lo, ln) in enumerate(xgroups):
                    hi = lo + ln
                    if tb == 0:
                        for gi, (s, l) in enumerate(wq_groups):
                            if lo <= s < hi:
                                nc.sync.dma_start(
                                    out=wq_g[gi],
                                    in_=wq_r[:, s * DQ : (s + l) * DQ],
                                )
                        if gx == min(2, len(xgroups) - 1):
                            # k/v weights are first needed after the q chains
                            nc.sync.dma_start(out=wk_all, in_=wk_r[:, :])
                            nc.sync.dma_start(out=wv_all, in_=wv_r[:, :])
                    xh = xin_p.tile([P, ln * QB], BF16, tag="xin", name="xh")
                    nc.sync.dma_start(out=xh, in_=xr[:, lo:hi, t0 : t0 + QB])
                    for c in range(ln):
                        xts.append(xh[:, c * QB : (c + 1) * QB])
                if tb == 0:
                    # consts are first needed by chunk-0 attention; keep them
                    # out of the warmup-critical DMA window
                    nc.sync.dma_start(out=ones_sb, in_=consts_t[0])
                    nc.sync.dma_start(out=tri_sb, in_=consts_t[1])
                if tb == 1:
                    # o_proj weights are first needed by the chunk-0 drains
                    for g in range(NWOG):
                        w = min(2, NHC - 2 * g) * H
                        nc.sync.dma_start(
                            out=wo_all[g],
                            in_=wo_rr[:, g * 2 * H : g * 2 * H + w],
                        )

                qtiles = []
                for dq in range(NHC):
                    q_ps = qs_p.tile([P, QB], F32, tag="qs", name="qps")
                    for hb in range(HB):
                        nc.tensor.matmul(
                            q_ps,
                            wq_sl(hb, dq * P, (dq + 1) * P),
                            xts[hb],
                            start=(hb == 0),
                            stop=(hb == HB - 1),
                        )
                    qt = qt_p.tile([P, QB], BF16, tag="qt", name="qt")
                    rope(qt, q_ps, cos_sb, sinx_sb)
                    qtiles.append(qt)
                k_ps = qs_p.tile([P, QB], F32, tag="qs", name="kps")
                for hb in range(HB):
                    nc.tensor.matmul(
                        k_ps,
                        wk_all[:, hb * P : (hb + 1) * P],
                        xts[hb],
                        start=(hb == 0),
                        stop=(hb == HB - 1),
                    )
                rope(kT[:, t0 : t0 + QB], k_ps, cos_sb, sinx_sb)
                for i in range(QB // P):
                    v_ps = psv_p.tile([P, P], F32, tag="vps")
                    for hb in range(HB):
                        nc.tensor.matmul(
                            v_ps,
                            xts[hb][:, i * P : (i + 1) * P],
                            wv_all[:, hb * P : (hb + 1) * P],
                            start=(hb == 0),
                            stop=(hb == HB - 1),
                        )
                    nc.scalar.copy(vt[tb * (QB // P) + i], v_ps)
                return qtiles

            def emit_attn(b, qb, qtiles, save_drains=False):
                """NHC heads of attention for q tokens [qb*QB,(qb+1)*QB) of
                batch b -> list of attn tiles [128 d, QB q] (bf16).
                save_drains: drain at half rate so micro-ops carry over as
                PE filler for the (larger, drain-starved) next chunk."""
                attn_tiles = []
                nkb = (qb + 1) * RB
                def emit_S(h, kb):
                    k0 = b * S + kb * KB
                    o = kb - qb * RB
                    c0 = max(o, 0) * KB
                    s_ps = qs_p.tile([P, QB], F32, tag="qs", name="sps")
                    nc.tensor.matmul(
                        s_ps[:, c0:QB],
                        kT[:, k0 : k0 + KB],
                        qtiles[h][:, c0:QB],
                        start=True,
                        stop=True,
                    )
                    return s_ps

                for h in range(NHC):
                    o_ps = pso_p.tile([P, QB], F32, tag="ops")
                    la0 = lacc_p.tile([P, QB], BF16, tag="la0", name="la0")
                    la1 = lacc_p.tile([P, QB], BF16, tag="la1", name="la1")
                    la = [la0, la1]
                    s_cur = emit_S(h, 0)
                    for kb in range(nkb):
                        k0 = b * S + kb * KB
                        o = kb - qb * RB  # >=0 on the diagonal supertile
                        c0 = max(o, 0) * KB  # first q column attending to kb
                        p_sb = p_p.tile([P, QB], BF16, tag="P")
                        if o > 0:
                            # columns below the diagonal never attend to kb
                            nc.gpsimd.memset(p_sb[:, 0:c0], 0.0)
                        nc.scalar.activation(
                            p_sb[:, c0:QB],
                            s_cur[:, c0:QB],
                            mybir.ActivationFunctionType.Exp,
                            scale=scale,
                        )
                        # software-pipeline: next S between exp and O so the
                        # PE has exp-independent work in the wait window
                        if kb + 1 < nkb:
                            s_cur = emit_S(h, kb + 1)
                        elif h + 1 < NHC:
                            s_cur = None
                        if o >= 0:
                            nc.vector.tensor_tensor(
                                p_sb[:, c0 : c0 + KB],
                                p_sb[:, c0 : c0 + KB],
                                tri_sb,
                                mybir.AluOpType.mult,
                            )
                        nc.tensor.matmul(
                            o_ps,
                            vt[k0 // P],
                            p_sb,
                            start=(kb == 0),
                            stop=(kb == nkb - 1),
                        )
                        # softmax denominator: two bf16 accumulators on DVE
                        if kb < 2:
                            nc.vector.tensor_copy(la[kb], p_sb)
                        else:
                            nc.vector.tensor_tensor(
                                la[kb & 1], la[kb & 1], p_sb, mybir.AluOpType.add
                            )
                        if not (save_drains and kb % 2 == 1):
                            drain(1)
                    # combine accumulators on DVE, then one tiny PE matmul
                    # broadcasts the partition-sum of L into PSUM
                    nc.vector.tensor_tensor(la0, la0, la1, mybir.AluOpType.add)
                    l_ps = qs_p.tile([P, QB], F32, tag="qs", name="lps")
                    nc.tensor.matmul(l_ps, ones_sb, la0, start=True, stop=True)
                    rinv = rt2_pool.tile([P, QB], F32, tag="rinv")
                    nc.vector.reciprocal_approx_fast(out=rinv, in_=l_ps)
                    attn_sb = ao_p.tile([P, QB], BF16, tag="attn")
                    nc.vector.tensor_tensor(
                        attn_sb, o_ps, rinv, mybir.AluOpType.mult
                    )
                    attn_tiles.append(attn_sb)
                    drain(1)
                return attn_tiles

            def push_oproj(b, qb, attn_tiles, fine_dma=False):
                """Queue partial o_proj micro-ops for chunk (b, qb):
                y[t0+ti*P, :] = sum_h attn_h[:, ti]^T @ wo_rows_h.
                fine_dma: write out each 512-col chunk as soon as it is
                copied (shrinks the end-of-kernel DMA drain)."""
                t0 = b * S + qb * QB
                HGRP = max(1, NHC // 2)  # heads per micro-op (half chains)
                for ti in range(QB // P):
                    holder = {}
                    for nch in range(H // QB):
                        for hg in range(0, NHC, HGRP):

                            def op(ti=ti, nch=nch, hg=hg, holder=holder,
                                   tiles=attn_tiles, t0=t0):
                                if nch == 0 and hg == 0:
                                    holder["y"] = y_p.tile(
                                        [P, H], BF16, tag="ysb", name="ysb"
                                    )
                                if hg == 0:
                                    holder["ps"] = psy_p.tile(
                                        [P, QB], F32, tag="yps", name="yps"
                                    )
                                y_sb = holder["y"]
                                y_ps = holder["ps"]
                                for h in range(hg, hg + HGRP):
                                    nc.tensor.matmul(
                                        y_ps,
                                        tiles[h][:, ti * P : (ti + 1) * P],
                                        wo_sl(h, nch * QB, (nch + 1) * QB),
                                        start=(h == 0),
                                        stop=(h == NHC - 1),
                                    )
                                if hg + HGRP < NHC:
                                    return
                                dst = y_sb[:, nch * QB : (nch + 1) * QB]
                                if nch % 4 != 3:
                                    nc.scalar.copy(dst, y_ps)
                                else:
                                    nc.vector.tensor_copy(dst, y_ps)
                                if fine_dma:
                                    nc.sync.dma_start(
                                        out=y_c[
                                            t0 + ti * P : t0 + (ti + 1) * P,
                                            nch * QB : (nch + 1) * QB,
                                        ],
                                        in_=dst,
                                    )
                                elif nch == H // QB - 1:
                                    nc.sync.dma_start(
                                        out=y_c[t0 + ti * P : t0 + (ti + 1) * P, :],
                                        in_=y_sb,
                                    )

                            microq.append(op)

            chunks = [(b, qb) for b in range(B) for qb in range(QBB)]
            for i, (b, qb) in enumerate(chunks):
                qtiles = emit_proj(i)
                atiles = emit_attn(
                    b, qb, qtiles, save_drains=(i == len(chunks) - 2)
                )
                push_oproj(b, qb, atiles, fine_dma=(i == len(chunks) - 1))
            drain(len(microq))
    nc.finalize()
    return nc


def _prep_inputs(hidden_states, wq, wk, wv, wo, position_ids, B, S, H, NH, NKV):
    """Host-side: bf16 casts, x transpose, rope tables, masks, per-core slices."""
    BT = B * S
    NHC = NH // NCORES
    DQ = NHC * P
    RB = QB // KB

    bf = ml_dtypes.bfloat16
    HB = H // P
    xr = np.ascontiguousarray(
        np.asarray(hidden_states)
        .reshape(BT, HB, P)
        .transpose(2, 1, 0)
    ).astype(bf)
    wq_b, wk_b, wv_b, wo_b = (np.asarray(w).astype(bf) for w in (wq, wk, wv, wo))

    def flat128(w):
        # [nb*128, D] -> [128, nb*D]: row (c*128+p) col j -> [p, c*D+j]
        nb, d = w.shape[0] // P, w.shape[1]
        return np.ascontiguousarray(
            w.reshape(nb, P, d).transpose(1, 0, 2).reshape(P, nb * d)
        )

    # rope tables in transposed layout [128 d, BT t]
    half = P // 2
    inv_freq = 1.0 / (10000.0 ** (np.arange(half, dtype=np.float64) / half))
    pos = np.asarray(position_ids).astype(np.float64).reshape(BT)
    ang = pos[None, :] * inv_freq[:, None]  # [64, BT]
    cos_t = np.concatenate([np.cos(ang), np.cos(ang)], 0).astype(np.float32)
    sinx_t = np.concatenate([-np.sin(ang), np.sin(ang)], 0).astype(np.float32)

    # consts: [ones for L row-sums, triangular causal mask tri[k,q]=k<=q]
    tri = np.arange(P)[:, None] <= np.arange(P)[None, :]
    consts_t = np.stack([np.ones((P, P)), tri]).astype(bf)

    in_maps = []
    for c in range(NCORES):
        in_maps.append(
            {
                "xr": xr,
                "wq_r": flat128(wq_b[:, c * DQ : (c + 1) * DQ]),
                "wk_r": flat128(wk_b[:, c * P : (c + 1) * P]),
                "wv_r": flat128(wv_b[:, c * P : (c + 1) * P]),
                "wo_rr": flat128(wo_b[c * DQ : (c + 1) * DQ, :]),
                "cos_t": cos_t,
                "sinx_t": sinx_t,
                "consts_t": consts_t,
            }
        )
    return in_maps


def run(hidden_states, wq, wk, wv, wo, position_ids, B, S, H, NH, NKV, trace=False):
    key = (B, S, H, NH, NKV)
    if key not in _CACHE:
        _CACHE[key] = build_program(B, S, H, NH, NKV)
    nc = _CACHE[key]
    in_maps = _prep_inputs(
        hidden_states, wq, wk, wv, wo, position_ids, B, S, H, NH, NKV
    )
    res = run_bass_kernel_spmd(nc, in_maps, core_ids=list(range(NCORES)), trace=trace)
    acc = np.zeros((B * S, H), np.float32)
    for c in range(NCORES):
        acc += np.asarray(res.results[c]["y_c"], dtype=np.float32)
    out = acc.reshape(B, S, H)
    return (out, res) if trace else (out, None)


def kernel(hidden_states, wq, wk, wv, wo, position_ids):
    out, _ = run(
        hidden_states, wq, wk, wv, wo, position_ids, 2, 2048, 4096, 32, 8
    )
    return out


# revision 74
# speedup vs baseline: 1.1811x; 1.1811x over previous
"""Tensor-parallel LlamaAttention (GQA + RoPE + causal) for 8 trn2 NeuronCores.

Sharding: column-parallel q/k/v by head (4 q-heads, 1 kv-head per core),
attention computed locally per head, then ROW-parallel o_proj on the local
head slice producing a partial output y_part[BT, H]; the host sums the 8
partials (the "all-reduce" of the RowParallel structure is done at gather
time on the host, so no on-device collective is needed).

Layout strategy (token dim = free dim, everything transposed):
  xr[p, c, t]  host-pre-transposed x (= x[t, c*128+p], bf16); weights are
               likewise host-flattened to [128, ...] so each matrix loads
               in a handful of big DMAs
  qT/kT[d, t]  from projection matmuls (lhsT=W block, rhs=x block)
  v[t, d]      natural layout (lhsT=x block, rhs=Wv block)
  S^T[k, q]    = matmul(lhsT=kT slice, rhs=qT slice)
  P^T          = exp(scale*S^T) via ACT (scores bounded ~|8|, exp safe)
  O^T[d, q]   += matmul(lhsT=v tile, rhs=P^T)    (PSUM accum over k)
  L (denoms)   = two bf16 DVE accumulators of P tiles, then a tiny
                 ones-matmul on PE broadcasts the partition-sum into PSUM
  attnT        = O^T * reciprocal_approx_fast(L) (SBUF, bf16)
  y_part      += matmul(lhsT=attnT slice, rhs=wo rows) -> DRAM, host-summed

Pipelining: projection chunk i (512 tokens), attention chunk i, and o_proj
micro-ops for chunk i-1 are interleaved in the instruction stream, so the
PE always has independent work during DMA warmup and exp-latency stalls.
"""

import math
import sys

import numpy as np

sys.path.insert(0, "/opt/trn_rl_repo")

import ml_dtypes  # noqa: E402

from concourse import bacc, mybir, tile  # noqa: E402
from concourse.bass_utils import run_bass_kernel_spmd  # noqa: E402

F32 = mybir.dt.float32
BF16 = mybir.dt.bfloat16
NCORES = 8
P = 128  # partitions / head dim
QB = 512  # q-block (PSUM free dim)
KB = 128  # k-block (contraction tile)

_CACHE = {}


def build_program(B, S, H, NH, NKV):
    nc = bacc.Bacc("TRN2", num_devices=NCORES)

    BT = B * S  # total tokens (4096)
    NHC = NH // NCORES  # q heads per core (4)
    DQ = NHC * P  # per-core q width (512)
    HB = H // P  # h blocks (32)
    TB = BT // QB  # token super-blocks (8)
    QBB = S // QB  # q blocks per batch (4)
    RB = QB // KB  # diag tiles per q block (4)

    # weights host-prearranged to [128 partitions, flat] so each matrix
    # loads with a handful of big DMAs (dma_start issue costs ~565ns each
    # on the sync sequencer; per-tile loads serialize the warmup)
    # x host-prearranged to [128, HB, BT]: xr[p, c, t] = x[t, c*128 + p]
    xr = nc.declare_dram_parameter("xr", [P, HB, BT], BF16, isOutput=False)
    wq_r = nc.declare_dram_parameter("wq_r", [P, HB * DQ], BF16, isOutput=False)
    wk_r = nc.declare_dram_parameter("wk_r", [P, HB * P], BF16, isOutput=False)
    wv_r = nc.declare_dram_parameter("wv_r", [P, HB * P], BF16, isOutput=False)
    wo_rr = nc.declare_dram_parameter("wo_rr", [P, NHC * H], BF16, isOutput=False)
    cos_t = nc.declare_dram_parameter("cos_t", [P, BT], F32, isOutput=False)
    sinx_t = nc.declare_dram_parameter("sinx_t", [P, BT], F32, isOutput=False)
    consts_t = nc.declare_dram_parameter("consts_t", [2, P, P], BF16, isOutput=False)
    y_c = nc.declare_dram_parameter("y_c", [BT, H], BF16, isOutput=True)

    scale = 1.0 / math.sqrt(P)

    with tile.TileContext(nc) as tc:
        with (
            tc.tile_pool(name="const", bufs=1) as constp,
            tc.tile_pool(name="persist", bufs=1) as persist,
            tc.tile_pool(name="qt", bufs=2 * NHC + 1) as qt_p,
            tc.tile_pool(name="xin", bufs=6) as xin_p,
            tc.tile_pool(name="wqkv", bufs=1) as w_p,
            tc.tile_pool(name="tabs", bufs=1) as tab_p,
            tc.tile_pool(name="ropetmp", bufs=1) as rt_p,
            tc.tile_pool(name="pP", bufs=6) as p_p,
            tc.tile_pool(name="lacc", bufs=2) as lacc_p,
            tc.tile_pool(name="rt2", bufs=3) as rt2_pool,
            tc.tile_pool(name="aout", bufs=3 * NHC + 2) as ao_p,
            tc.tile_pool(name="yout", bufs=2) as y_p,
            tc.tile_pool(name="psQS", bufs=2, space="PSUM") as qs_p,
            tc.tile_pool(name="psO", bufs=2, space="PSUM") as pso_p,
            tc.tile_pool(name="psV", bufs=2, space="PSUM") as psv_p,
            tc.tile_pool(name="psY", bufs=2, space="PSUM") as psy_p,
        ):
            # ones matrix for the L (softmax denominator) row-sum matmuls
            ones_sb = constp.tile([P, P], BF16, tag="ones")
            # triangular causal mask for the diagonal subtile: tri[k,q]=k<=q
            tri_sb = constp.tile([P, P], BF16, tag="tri")

            # persistent per-core activations (bf16)
            kT = persist.tile([P, BT], BF16, tag="kT")
            vt = [
                persist.tile([P, P], BF16, tag=f"v{i}", name=f"v{i}")
                for i in range(BT // P)
            ]
            # o_proj weights (rows for this core's heads), resident whole run
            # wo_all[g][:, (h%2)*H + j] = wo[head 2g+h row, col j]
            NWOG = (NHC + 1) // 2  # wo DMA groups of up to 2 heads
            wo_all = [
                persist.tile(
                    [P, min(2, NHC - 2 * g) * H], BF16, tag=f"wo{g}", name=f"wo{g}"
                )
                for g in range(NWOG)
            ]

            def wo_sl(h, c0, c1):
                return wo_all[h // 2][:, (h % 2) * H + c0 : (h % 2) * H + c1]

            # q/k/v projection weights, resident whole run, flat layout:
            # group g covers hb [s, s+l): tile[:, (hb-s)*DQ + j] = wq[hb*128+p, j]
            # (first group small so the very first q chain starts ~immediately)
            wq_groups = [(0, min(2, HB))]
            while sum(l for _, l in wq_groups) < HB:
                s = sum(l for _, l in wq_groups)
                wq_groups.append((s, min(6 if s == 2 else 8, HB - s)))
            wq_g = [
                w_p.tile([P, l * DQ], BF16, tag=f"wq{gi}", name=f"wq{gi}")
                for gi, (s, l) in enumerate(wq_groups)
            ]
            wk_all = w_p.tile([P, HB * P], BF16, tag="wk_all")
            wv_all = w_p.tile([P, HB * P], BF16, tag="wv_all")

            def wq_sl(hb, c0, c1):
                for gi, (s, l) in enumerate(wq_groups):
                    if s <= hb < s + l:
                        return wq_g[gi][:, (hb - s) * DQ + c0 : (hb - s) * DQ + c1]

            microq = []  # pending o_proj micro-ops

            def drain(n):
                for _ in range(min(n, len(microq))):
                    microq.pop(0)()

            def rope(dst, ps, cos_sb, sinx_sb):
                """dst = ps*cos + shift64(ps)*sinx (all [128,QB])"""
                t1 = rt_p.tile([P, QB], F32, tag="ropet1")
                t2 = rt_p.tile([P, QB], F32, tag="ropet2")
                nc.vector.tensor_tensor(t1, ps, cos_sb, mybir.AluOpType.mult)
                hh = P // 2
                nc.vector.tensor_tensor(
                    t2[0:hh], ps[hh:P], sinx_sb[0:hh], mybir.AluOpType.mult
                )
                nc.vector.tensor_tensor(
                    t2[hh:P], ps[0:hh], sinx_sb[hh:P], mybir.AluOpType.mult
                )
                nc.vector.tensor_tensor(dst, t1, t2, mybir.AluOpType.add)

            def emit_proj(tb):
                """Projections + rope for token block tb -> per-head q tiles."""
                t0 = tb * QB
                cos_sb = tab_p.tile([P, QB], F32, tag="cos")
                sinx_sb = tab_p.tile([P, QB], F32, tag="sinx")
                nc.sync.dma_start(out=cos_sb, in_=cos_t[:, t0 : t0 + QB])
                nc.sync.dma_start(out=sinx_sb, in_=sinx_t[:, t0 : t0 + QB])

                # stream x tiles; on tb 0 the x groups mirror the staged wq
                # groups (first group tiny) and interleave the weight loads,
                # so the first q chain starts within a few us
                xts = []
                XG = (HB + 3) // 4  # hb blocks per x DMA group (steady state)
                if tb == 0:
                    xgroups = list(wq_groups)
                else:
                    xgroups = [
                        (g * XG, min((g + 1) * XG, HB) - g * XG)
                        for g in range(4)
                        if g * XG < HB
                    ]
                for gx, (lo, ln) in enumerate(xgroups):
                    hi = lo + ln
                    if tb == 0:
                        for gi, (s, l) in enumerate(wq_groups):
                            if lo <= s < hi:
                                nc.sync.dma_start(
                                    out=wq_g[gi],
                                    in_=wq_r[:, s * DQ : (s + l) * DQ],
                                )
                        if gx == min(2, len(xgroups) - 1):
                            # k/v weights are first needed after the q chains
                            nc.sync.dma_start(out=wk_all, in_=wk_r[:, :])
                            nc.sync.dma_start(out=wv_all, in_=wv_r[:, :])
                    xh = xin_p.tile([P, ln * QB], BF16, tag="xin", name="xh")
                    nc.sync.dma_start(out=xh, in_=xr[:, lo:hi, t0 : t0 + QB])
                    for c in range(ln):
                        xts.append(xh[:, c * QB : (c + 1) * QB])
                if tb == 0:
                    # consts are first needed by chunk-0 attention; keep them
                    # out of the warmup-critical DMA window
                    nc.sync.dma_start(out=ones_sb, in_=consts_t[0])
                    nc.sync.dma_start(out=tri_sb, in_=consts_t[1])
                if tb == 1:
                    # o_proj weights are first needed by the chunk-0 drains
                    for g in range(NWOG):
                        w = min(2, NHC - 2 * g) * H
                        nc.sync.dma_start(
                            out=wo_all[g],
                            in_=wo_rr[:, g * 2 * H : g * 2 * H + w],
                        )

                qtiles = []
                for dq in range(NHC):
                    q_ps = qs_p.tile([P, QB], F32, tag="qs", name="qps")
                    for hb in range(HB):
                        nc.tensor.matmul(
                            q_ps,
                            wq_sl(hb, dq * P, (dq + 1) * P),
                            xts[hb],
                            start=(hb == 0),
                            stop=(hb == HB - 1),
                        )
                    qt = qt_p.tile([P, QB], BF16, tag="qt", name="qt")
                    rope(qt, q_ps, cos_sb, sinx_sb)
                    qtiles.append(qt)
                k_ps = qs_p.tile([P, QB], F32, tag="qs", name="kps")
                for hb in range(HB):
                    nc.tensor.matmul(
                        k_ps,
                        wk_all[:, hb * P : (hb + 1) * P],
                        xts[hb],
                        start=(hb == 0),
                        stop=(hb == HB - 1),
                    )
                rope(kT[:, t0 : t0 + QB], k_ps, cos_sb, sinx_sb)
                for i in range(QB // P):
                    v_ps = psv_p.tile([P, P], F32, tag="vps")
                    for hb in range(HB):
                        nc.tensor.matmul(
                            v_ps,
                            xts[hb][:, i * P : (i + 1) * P],
                            wv_all[:, hb * P : (hb + 1) * P],
                            start=(hb == 0),
                            stop=(hb == HB - 1),
                        )
                    nc.scalar.copy(vt[tb * (QB // P) + i], v_ps)
                return qtiles

            def emit_attn(b, qb, qtiles, save_drains=False):
                """NHC heads of attention for q tokens [qb*QB,(qb+1)*QB) of
                batch b -> list of attn tiles [128 d, QB q] (bf16).
                save_drains: drain at half rate so micro-ops carry over as
                PE filler for the (larger, drain-starved) next chunk."""
                attn_tiles = []
                nkb = (qb + 1) * RB
                def emit_S(h, kb):
                    k0 = b * S + kb * KB
                    o = kb - qb * RB
                    c0 = max(o, 0) * KB
                    s_ps = qs_p.tile([P, QB], F32, tag="qs", name="sps")
                    nc.tensor.matmul(
                        s_ps[:, c0:QB],
                        kT[:, k0 : k0 + KB],
                        qtiles[h][:, c0:QB],
                        start=True,
                        stop=True,
                    )
                    return s_ps

                for h in range(NHC):
                    o_ps = pso_p.tile([P, QB], F32, tag="ops")
                    la0 = lacc_p.tile([P, QB], BF16, tag="la0", name="la0")
                    la1 = lacc_p.tile([P, QB], BF16, tag="la1", name="la1")
                    la = [la0, la1]
                    s_cur = emit_S(h, 0)
                    for kb in range(nkb):
                        k0 = b * S + kb * KB
                        o = kb - qb * RB  # >=0 on the diagonal supertile
                        c0 = max(o, 0) * KB  # first q column attending to kb
                        p_sb = p_p.tile([P, QB], BF16, tag="P")
                        if o > 0:
                            # columns below the diagonal never attend to kb
                            nc.gpsimd.memset(p_sb[:, 0:c0], 0.0)
                        nc.scalar.activation(
                            p_sb[:, c0:QB],
                            s_cur[:, c0:QB],
                            mybir.ActivationFunctionType.Exp,
                            scale=scale,
                        )
                        # software-pipeline: next S between exp and O so the
                        # PE has exp-independent work in the wait window
                        if kb + 1 < nkb:
                            s_cur = emit_S(h, kb + 1)
                        elif h + 1 < NHC:
                            s_cur = None
                        if o >= 0:
                            nc.vector.tensor_tensor(
                                p_sb[:, c0 : c0 + KB],
                                p_sb[:, c0 : c0 + KB],
                                tri_sb,
                                mybir.AluOpType.mult,
                            )
                        nc.tensor.matmul(
                            o_ps,
                            vt[k0 // P],
                            p_sb,
                            start=(kb == 0),
                            stop=(kb == nkb - 1),
                        )
                        # softmax denominator: two bf16 accumulators on DVE
                        if kb < 2:
                            nc.vector.tensor_copy(la[kb], p_sb)
                        else:
                            nc.vector.tensor_tensor(
                                la[kb & 1], la[kb & 1], p_sb, mybir.AluOpType.add
                            )
                        if not (save_drains and kb % 2 == 1):
                            drain(1)
                    # combine accumulators on DVE, then one tiny PE matmul
                    # broadcasts the partition-sum of L into PSUM
                    nc.vector.tensor_tensor(la0, la0, la1, mybir.AluOpType.add)
                    l_ps = qs_p.tile([P, QB], F32, tag="qs", name="lps")
                    nc.tensor.matmul(l_ps, ones_sb, la0, start=True, stop=True)
                    rinv = rt2_pool.tile([P, QB], F32, tag="rinv")
                    nc.vector.reciprocal_approx_fast(out=rinv, in_=l_ps)
                    attn_sb = ao_p.tile([P, QB], BF16, tag="attn")
                    nc.vector.tensor_tensor(
                        attn_sb, o_ps, rinv, mybir.AluOpType.mult
                    )
                    attn_tiles.append(attn_sb)
                    drain(1)
                return attn_tiles

            def push_oproj(b, qb, attn_tiles, fine_dma=False):
                """Queue partial o_proj micro-ops for chunk (b, qb):
                y[t0+ti*P, :] = sum_h attn_h[:, ti]^T @ wo_rows_h.
                fine_dma: write out each 512-col chunk as soon as it is
                copied (shrinks the end-of-kernel DMA drain)."""
                t0 = b * S + qb * QB
                for ti in range(QB // P):
                    holder = {}
                    for nch in range(H // QB):

                        def op(ti=ti, nch=nch, holder=holder, tiles=attn_tiles, t0=t0):
                            if nch == 0:
                                holder["y"] = y_p.tile(
                                    [P, H], BF16, tag="ysb", name="ysb"
                                )
                            y_sb = holder["y"]
                            y_ps = psy_p.tile([P, QB], F32, tag="yps")
                            for h in range(NHC):
                                nc.tensor.matmul(
                                    y_ps,
                                    tiles[h][:, ti * P : (ti + 1) * P],
                                    wo_sl(h, nch * QB, (nch + 1) * QB),
                                    start=(h == 0),
                                    stop=(h == NHC - 1),
                                )
                            dst = y_sb[:, nch * QB : (nch + 1) * QB]
                            if nch % 4 != 3:
                                nc.scalar.copy(dst, y_ps)
                            else:
                                nc.vector.tensor_copy(dst, y_ps)
                            if fine_dma:
                                nc.sync.dma_start(
                                    out=y_c[
                                        t0 + ti * P : t0 + (ti + 1) * P,
                                        nch * QB : (nch + 1) * QB,
                                    ],
                                    in_=dst,
                                )
                            elif nch == H // QB - 1:
                                nc.sync.dma_start(
                                    out=y_c[t0 + ti * P : t0 + (ti + 1) * P, :],
                                    in_=y_sb,
                                )

                        microq.append(op)

            chunks = [(b, qb) for b in range(B) for qb in range(QBB)]
            for i, (b, qb) in enumerate(chunks):
                qtiles = emit_proj(i)
                atiles = emit_attn(
                    b, qb, qtiles, save_drains=(i == len(chunks) - 2)
                )
                push_oproj(b, qb, atiles, fine_dma=(i == len(chunks) - 1))
            drain(len(microq))
    nc.finalize()
    return nc


def _prep_inputs(hidden_states, wq, wk, wv, wo, position_ids, B, S, H, NH, NKV):
    """Host-side: bf16 casts, x transpose, rope tables, masks, per-core slices."""
    BT = B * S
    NHC = NH // NCORES
    DQ = NHC * P
    RB = QB // KB

    bf = ml_dtypes.bfloat16
    HB = H // P
    xr = np.ascontiguousarray(
        np.asarray(hidden_states)
        .reshape(BT, HB, P)
        .transpose(2, 1, 0)
    ).astype(bf)
    wq_b, wk_b, wv_b, wo_b = (np.asarray(w).astype(bf) for w in (wq, wk, wv, wo))

    def flat128(w):
        # [nb*128, D] -> [128, nb*D]: row (c*128+p) col j -> [p, c*D+j]
        nb, d = w.shape[0] // P, w.shape[1]
        return np.ascontiguousarray(
            w.reshape(nb, P, d).transpose(1, 0, 2).reshape(P, nb * d)
        )

    # rope tables in transposed layout [128 d, BT t]
    half = P // 2
    inv_freq = 1.0 / (10000.0 ** (np.arange(half, dtype=np.float64) / half))
    pos = np.asarray(position_ids).astype(np.float64).reshape(BT)
    ang = pos[None, :] * inv_freq[:, None]  # [64, BT]
    cos_t = np.concatenate([np.cos(ang), np.cos(ang)], 0).astype(np.float32)
    sinx_t = np.concatenate([-np.sin(ang), np.sin(ang)], 0).astype(np.float32)

    # consts: [ones for L row-sums, triangular causal mask tri[k,q]=k<=q]
    tri = np.arange(P)[:, None] <= np.arange(P)[None, :]
    consts_t = np.stack([np.ones((P, P)), tri]).astype(bf)

    in_maps = []
    for c in range(NCORES):
        in_maps.append(
            {
                "xr": xr,
                "wq_r": flat128(wq_b[:, c * DQ : (c + 1) * DQ]),
                "wk_r": flat128(wk_b[:, c * P : (c + 1) * P]),
                "wv_r": flat128(wv_b[:, c * P : (c + 1) * P]),
                "wo_rr": flat128(wo_b[c * DQ : (c + 1) * DQ, :]),
                "cos_t": cos_t,
                "sinx_t": sinx_t,
                "consts_t": consts_t,
            }
        )
    return in_maps


def run(hidden_states, wq, wk, wv, wo, position_ids, B, S, H, NH, NKV, trace=False):
    key = (B, S, H, NH, NKV)
    if key not in _CACHE:
        _CACHE[key] = build_program(B, S, H, NH, NKV)
    nc = _CACHE[key]
    in_maps = _prep_inputs(
        hidden_states, wq, wk, wv, wo, position_ids, B, S, H, NH, NKV
    )
    res = run_bass_kernel_spmd(nc, in_maps, core_ids=list(range(NCORES)), trace=trace)
    acc = np.zeros((B * S, H), np.float32)
    for c in range(NCORES):
        acc += np.asarray(res.results[c]["y_c"], dtype=np.float32)
    out = acc.reshape(B, S, H)
    return (out, res) if trace else (out, None)


def kernel(hidden_states, wq, wk, wv, wo, position_ids):
    out, _ = run(
        hidden_states, wq, wk, wv, wo, position_ids, 2, 2048, 4096, 32, 8
    )
    return out


# revision 75
# speedup vs baseline: 1.2078x; 1.0226x over previous
"""Tensor-parallel LlamaAttention (GQA + RoPE + causal) for 8 trn2 NeuronCores.

Sharding: column-parallel q/k/v by head (4 q-heads, 1 kv-head per core),
attention computed locally per head, then ROW-parallel o_proj on the local
head slice producing a partial output y_part[BT, H]; the host sums the 8
partials (the "all-reduce" of the RowParallel structure is done at gather
time on the host, so no on-device collective is needed).

Layout strategy (token dim = free dim, everything transposed):
  xr[p, c, t]  host-pre-transposed x (= x[t, c*128+p], bf16); weights are
               likewise host-flattened to [128, ...] so each matrix loads
               in a handful of big DMAs
  qT/kT[d, t]  from projection matmuls (lhsT=W block, rhs=x block)
  v[t, d]      natural layout (lhsT=x block, rhs=Wv block)
  S^T[k, q]    = matmul(lhsT=kT slice, rhs=qT slice)
  P^T          = exp(scale*S^T) via ACT (scores bounded ~|8|, exp safe)
  O^T[d, q]   += matmul(lhsT=v tile, rhs=P^T)    (PSUM accum over k)
  L (denoms)   = two bf16 DVE accumulators of P tiles, then a tiny
                 ones-matmul on PE broadcasts the partition-sum into PSUM
  attnT        = O^T * reciprocal_approx_fast(L) (SBUF, bf16)
  y_part      += matmul(lhsT=attnT slice, rhs=wo rows) -> DRAM, host-summed

Pipelining: projection chunk i (512 tokens), attention chunk i, and o_proj
micro-ops for chunk i-1 are interleaved in the instruction stream, so the
PE always has independent work during DMA warmup and exp-latency stalls.
"""

import math
import sys

import numpy as np

sys.path.insert(0, "/opt/trn_rl_repo")

import ml_dtypes  # noqa: E402

from concourse import bacc, mybir, tile  # noqa: E402
from concourse.bass_utils import run_bass_kernel_spmd  # noqa: E402

F32 = mybir.dt.float32
BF16 = mybir.dt.bfloat16
NCORES = 8
P = 128  # partitions / head dim
QB = 512  # q-block (PSUM free dim)
KB = 128  # k-block (contraction tile)

_CACHE = {}


def build_program(B, S, H, NH, NKV):
    nc = bacc.Bacc("TRN2", num_devices=NCORES)

    BT = B * S  # total tokens (4096)
    NHC = NH // NCORES  # q heads per core (4)
    DQ = NHC * P  # per-core q width (512)
    HB = H // P  # h blocks (32)
    TB = BT // QB  # token super-blocks (8)
    QBB = S // QB  # q blocks per batch (4)
    RB = QB // KB  # diag tiles per q block (4)

    # weights host-prearranged to [128 partitions, flat] so each matrix
    # loads with a handful of big DMAs (dma_start issue costs ~565ns each
    # on the sync sequencer; per-tile loads serialize the warmup)
    # x host-prearranged to [128, HB, BT]: xr[p, c, t] = x[t, c*128 + p]
    xr = nc.declare_dram_parameter("xr", [P, HB, BT], BF16, isOutput=False)
    wq_r = nc.declare_dram_parameter("wq_r", [P, HB * DQ], BF16, isOutput=False)
    wk_r = nc.declare_dram_parameter("wk_r", [P, HB * P], BF16, isOutput=False)
    wv_r = nc.declare_dram_parameter("wv_r", [P, HB * P], BF16, isOutput=False)
    wo_rr = nc.declare_dram_parameter("wo_rr", [P, NHC * H], BF16, isOutput=False)
    cos_t = nc.declare_dram_parameter("cos_t", [P, BT], F32, isOutput=False)
    sinx_t = nc.declare_dram_parameter("sinx_t", [P, BT], F32, isOutput=False)
    consts_t = nc.declare_dram_parameter("consts_t", [2, P, P], BF16, isOutput=False)
    y_c = nc.declare_dram_parameter("y_c", [BT, H], BF16, isOutput=True)

    scale = 1.0 / math.sqrt(P)

    with tile.TileContext(nc) as tc:
        with (
            tc.tile_pool(name="const", bufs=1) as constp,
            tc.tile_pool(name="persist", bufs=1) as persist,
            tc.tile_pool(name="qt", bufs=2 * NHC + 1) as qt_p,
            tc.tile_pool(name="xin", bufs=6) as xin_p,
            tc.tile_pool(name="wqkv", bufs=1) as w_p,
            tc.tile_pool(name="tabs", bufs=1) as tab_p,
            tc.tile_pool(name="ropetmp", bufs=1) as rt_p,
            tc.tile_pool(name="pP", bufs=6) as p_p,
            tc.tile_pool(name="lacc", bufs=2) as lacc_p,
            tc.tile_pool(name="rt2", bufs=3) as rt2_pool,
            tc.tile_pool(name="aout", bufs=3 * NHC + 2) as ao_p,
            tc.tile_pool(name="yout", bufs=2) as y_p,
            tc.tile_pool(name="psQS", bufs=2, space="PSUM") as qs_p,
            tc.tile_pool(name="psO", bufs=2, space="PSUM") as pso_p,
            tc.tile_pool(name="psV", bufs=2, space="PSUM") as psv_p,
            tc.tile_pool(name="psY", bufs=2, space="PSUM") as psy_p,
        ):
            # ones matrix for the L (softmax denominator) row-sum matmuls
            ones_sb = constp.tile([P, P], BF16, tag="ones")
            # triangular causal mask for the diagonal subtile: tri[k,q]=k<=q
            tri_sb = constp.tile([P, P], BF16, tag="tri")

            # persistent per-core activations (bf16)
            kT = persist.tile([P, BT], BF16, tag="kT")
            vt = [
                persist.tile([P, P], BF16, tag=f"v{i}", name=f"v{i}")
                for i in range(BT // P)
            ]
            # o_proj weights (rows for this core's heads), resident whole run
            # wo_all[g][:, (h%2)*H + j] = wo[head 2g+h row, col j]
            NWOG = (NHC + 1) // 2  # wo DMA groups of up to 2 heads
            wo_all = [
                persist.tile(
                    [P, min(2, NHC - 2 * g) * H], BF16, tag=f"wo{g}", name=f"wo{g}"
                )
                for g in range(NWOG)
            ]

            def wo_sl(h, c0, c1):
                return wo_all[h // 2][:, (h % 2) * H + c0 : (h % 2) * H + c1]

            # q/k/v projection weights, resident whole run, flat layout:
            # group g covers hb [s, s+l): tile[:, (hb-s)*DQ + j] = wq[hb*128+p, j]
            # (first group small so the very first q chain starts ~immediately)
            wq_groups = [(0, min(2, HB))]
            while sum(l for _, l in wq_groups) < HB:
                s = sum(l for _, l in wq_groups)
                wq_groups.append((s, min(6 if s == 2 else 8, HB - s)))
            wq_g = [
                w_p.tile([P, l * DQ], BF16, tag=f"wq{gi}", name=f"wq{gi}")
                for gi, (s, l) in enumerate(wq_groups)
            ]
            wk_all = w_p.tile([P, HB * P], BF16, tag="wk_all")
            wv_all = w_p.tile([P, HB * P], BF16, tag="wv_all")

            def wq_sl(hb, c0, c1):
                for gi, (s, l) in enumerate(wq_groups):
                    if s <= hb < s + l:
                        return wq_g[gi][:, (hb - s) * DQ + c0 : (hb - s) * DQ + c1]

            microq = []  # pending o_proj micro-ops

            def drain(n):
                for _ in range(min(n, len(microq))):
                    microq.pop(0)()

            def rope(dst, ps, cos_sb, sinx_sb):
                """dst = ps*cos + shift64(ps)*sinx (all [128,QB])"""
                t1 = rt_p.tile([P, QB], F32, tag="ropet1")
                t2 = rt_p.tile([P, QB], F32, tag="ropet2")
                nc.vector.tensor_tensor(t1, ps, cos_sb, mybir.AluOpType.mult)
                hh = P // 2
                nc.vector.tensor_tensor(
                    t2[0:hh], ps[hh:P], sinx_sb[0:hh], mybir.AluOpType.mult
                )
                nc.vector.tensor_tensor(
                    t2[hh:P], ps[0:hh], sinx_sb[hh:P], mybir.AluOpType.mult
                )
                nc.vector.tensor_tensor(dst, t1, t2, mybir.AluOpType.add)

            def emit_proj(tb):
                """Projections + rope for token block tb -> per-head q tiles."""
                t0 = tb * QB
                cos_sb = tab_p.tile([P, QB], F32, tag="cos")
                sinx_sb = tab_p.tile([P, QB], F32, tag="sinx")
                nc.sync.dma_start(out=cos_sb, in_=cos_t[:, t0 : t0 + QB])
                nc.sync.dma_start(out=sinx_sb, in_=sinx_t[:, t0 : t0 + QB])

                # stream x tiles; on tb 0 the x groups mirror the staged wq
                # groups (first group tiny) and interleave the weight loads,
                # so the first q chain starts within a few us
                xts = []
                XG = (HB + 3) // 4  # hb blocks per x DMA group (steady state)
                if tb == 0:
                    xgroups = list(wq_groups)
                else:
                    xgroups = [
                        (g * XG, min((g + 1) * XG, HB) - g * XG)
                        for g in range(4)
                        if g * XG < HB
                    ]
                for gx, (lo, ln) in enumerate(xgroups):
                    hi = lo + ln
                    if tb == 0:
                        for gi, (s, l) in enumerate(wq_groups):
                            if lo <= s < hi:
                                nc.sync.dma_start(
                                    out=wq_g[gi],
                                    in_=wq_r[:, s * DQ : (s + l) * DQ],
                                )
                        if gx == min(2, len(xgroups) - 1):
                            # k/v weights are first needed after the q chains
                            nc.sync.dma_start(out=wk_all, in_=wk_r[:, :])
                            nc.sync.dma_start(out=wv_all, in_=wv_r[:, :])
                    xh = xin_p.tile([P, ln * QB], BF16, tag="xin", name="xh")
                    nc.sync.dma_start(out=xh, in_=xr[:, lo:hi, t0 : t0 + QB])
                    for c in range(ln):
                        xts.append(xh[:, c * QB : (c + 1) * QB])
                if tb == 0:
                    # consts are first needed by chunk-0 attention; keep them
                    # out of the warmup-critical DMA window
                    nc.sync.dma_start(out=ones_sb, in_=consts_t[0])
                    nc.sync.dma_start(out=tri_sb, in_=consts_t[1])
                if tb == 1:
                    # o_proj weights are first needed by the chunk-0 drains
                    for g in range(NWOG):
                        w = min(2, NHC - 2 * g) * H
                        nc.sync.dma_start(
                            out=wo_all[g],
                            in_=wo_rr[:, g * 2 * H : g * 2 * H + w],
                        )

                qtiles = []
                for dq in range(NHC):
                    q_ps = qs_p.tile([P, QB], F32, tag="qs", name="qps")
                    for hb in range(HB):
                        nc.tensor.matmul(
                            q_ps,
                            wq_sl(hb, dq * P, (dq + 1) * P),
                            xts[hb],
                            start=(hb == 0),
                            stop=(hb == HB - 1),
                        )
                    qt = qt_p.tile([P, QB], BF16, tag="qt", name="qt")
                    rope(qt, q_ps, cos_sb, sinx_sb)
                    qtiles.append(qt)
                k_ps = qs_p.tile([P, QB], F32, tag="qs", name="kps")
                for hb in range(HB):
                    nc.tensor.matmul(
                        k_ps,
                        wk_all[:, hb * P : (hb + 1) * P],
                        xts[hb],
                        start=(hb == 0),
                        stop=(hb == HB - 1),
                    )
                rope(kT[:, t0 : t0 + QB], k_ps, cos_sb, sinx_sb)
                for i in range(QB // P):
                    v_ps = psv_p.tile([P, P], F32, tag="vps")
                    for hb in range(HB):
                        nc.tensor.matmul(
                            v_ps,
                            xts[hb][:, i * P : (i + 1) * P],
                            wv_all[:, hb * P : (hb + 1) * P],
                            start=(hb == 0),
                            stop=(hb == HB - 1),
                        )
                    nc.scalar.copy(vt[tb * (QB // P) + i], v_ps)
                return qtiles

            def emit_attn(b, qb, qtiles, save_drains=False):
                """NHC heads of attention for q tokens [qb*QB,(qb+1)*QB) of
                batch b -> list of attn tiles [128 d, QB q] (bf16).
                save_drains: drain at half rate so micro-ops carry over as
                PE filler for the (larger, drain-starved) next chunk."""
                attn_tiles = []
                nkb = (qb + 1) * RB
                def emit_S(h, kb):
                    k0 = b * S + kb * KB
                    o = kb - qb * RB
                    c0 = max(o, 0) * KB
                    s_ps = qs_p.tile([P, QB], F32, tag="qs", name="sps")
                    nc.tensor.matmul(
                        s_ps[:, c0:QB],
                        kT[:, k0 : k0 + KB],
                        qtiles[h][:, c0:QB],
                        start=True,
                        stop=True,
                    )
                    return s_ps

                for h in range(NHC):
                    o_ps = pso_p.tile([P, QB], F32, tag="ops")
                    la0 = lacc_p.tile([P, QB], BF16, tag="la0", name="la0")
                    la1 = lacc_p.tile([P, QB], BF16, tag="la1", name="la1")
                    la = [la0, la1]
                    s_cur = emit_S(h, 0)
                    for kb in range(nkb):
                        k0 = b * S + kb * KB
                        o = kb - qb * RB  # >=0 on the diagonal supertile
                        c0 = max(o, 0) * KB  # first q column attending to kb
                        p_sb = p_p.tile([P, QB], BF16, tag="P")
                        if o > 0:
                            # columns below the diagonal never attend to kb
                            nc.gpsimd.memset(p_sb[:, 0:c0], 0.0)
                        nc.scalar.activation(
                            p_sb[:, c0:QB],
                            s_cur[:, c0:QB],
                            mybir.ActivationFunctionType.Exp,
                            scale=scale,
                        )
                        # software-pipeline: next S between exp and O so the
                        # PE has exp-independent work in the wait window
                        if kb + 1 < nkb:
                            s_cur = emit_S(h, kb + 1)
                        elif h + 1 < NHC:
                            s_cur = None
                        if o >= 0:
                            nc.vector.tensor_tensor(
                                p_sb[:, c0 : c0 + KB],
                                p_sb[:, c0 : c0 + KB],
                                tri_sb,
                                mybir.AluOpType.mult,
                            )
                        nc.tensor.matmul(
                            o_ps,
                            vt[k0 // P],
                            p_sb,
                            start=(kb == 0),
                            stop=(kb == nkb - 1),
                        )
                        # softmax denominator: two bf16 accumulators on DVE
                        if kb < 2:
                            nc.vector.tensor_copy(la[kb], p_sb)
                        else:
                            nc.vector.tensor_tensor(
                                la[kb & 1], la[kb & 1], p_sb, mybir.AluOpType.add
                            )
                        if not (save_drains and kb % 2 == 1):
                            drain(1)
                    # combine accumulators on DVE, then one tiny PE matmul
                    # broadcasts the partition-sum of L into PSUM
                    nc.vector.tensor_tensor(la0, la0, la1, mybir.AluOpType.add)
                    l_ps = qs_p.tile([P, QB], F32, tag="qs", name="lps")
                    nc.tensor.matmul(l_ps, ones_sb, la0, start=True, stop=True)
                    rinv = rt2_pool.tile([P, QB], F32, tag="rinv")
                    nc.vector.reciprocal_approx_fast(out=rinv, in_=l_ps)
                    attn_sb = ao_p.tile([P, QB], BF16, tag="attn")
                    nc.vector.tensor_tensor(
                        attn_sb, o_ps, rinv, mybir.AluOpType.mult
                    )
                    attn_tiles.append(attn_sb)
                    drain(1)
                return attn_tiles

            def push_oproj(b, qb, attn_tiles, fine_dma=False):
                """Queue partial o_proj micro-ops for chunk (b, qb):
                y[t0+ti*P, :] = sum_h attn_h[:, ti]^T @ wo_rows_h.
                fine_dma: write out each 512-col chunk as soon as it is
                copied (shrinks the end-of-kernel DMA drain)."""
                t0 = b * S + qb * QB
                for ti in range(QB // P):
                    holder = {}
                    for nch in range(H // QB):

                        def op(ti=ti, nch=nch, holder=holder, tiles=attn_tiles, t0=t0):
                            if nch == 0:
                                holder["y"] = y_p.tile(
                                    [P, H], BF16, tag="ysb", name="ysb"
                                )
                            y_sb = holder["y"]
                            y_ps = psy_p.tile([P, QB], F32, tag="yps")
                            for h in range(NHC):
                                nc.tensor.matmul(
                                    y_ps,
                                    tiles[h][:, ti * P : (ti + 1) * P],
                                    wo_sl(h, nch * QB, (nch + 1) * QB),
                                    start=(h == 0),
                                    stop=(h == NHC - 1),
                                )
                            dst = y_sb[:, nch * QB : (nch + 1) * QB]
                            if nch % 4 != 3:
                                nc.scalar.copy(dst, y_ps)
                            else:
                                nc.vector.tensor_copy(dst, y_ps)
                            if fine_dma:
                                nc.sync.dma_start(
                                    out=y_c[
                                        t0 + ti * P : t0 + (ti + 1) * P,
                                        nch * QB : (nch + 1) * QB,
                                    ],
                                    in_=dst,
                                )
                            elif nch == H // QB - 1:
                                nc.sync.dma_start(
                                    out=y_c[t0 + ti * P : t0 + (ti + 1) * P, :],
                                    in_=y_sb,
                                )

                        microq.append(op)

            chunks = [(b, qb) for b in range(B) for qb in range(QBB)]
            # chunks with small causal extent finish their drains early while
            # the big (qb>=2) chunks are drain-starved; halve the drain rate
            # everywhere except the biggest chunks so micro-ops carry forward
            # into their exp-wait windows
            nch_ = len(chunks)
            for i, (b, qb) in enumerate(chunks):
                qtiles = emit_proj(i)
                atiles = emit_attn(
                    b, qb, qtiles,
                    save_drains=(qb < QBB - 1 and i != nch_ - 1 and i > 0),
                )
                push_oproj(b, qb, atiles, fine_dma=(i == nch_ - 1))
            drain(len(microq))
    nc.finalize()
    return nc


def _prep_inputs(hidden_states, wq, wk, wv, wo, position_ids, B, S, H, NH, NKV):
    """Host-side: bf16 casts, x transpose, rope tables, masks, per-core slices."""
    BT = B * S
    NHC = NH // NCORES
    DQ = NHC * P
    RB = QB // KB

    bf = ml_dtypes.bfloat16
    HB = H // P
    xr = np.ascontiguousarray(
        np.asarray(hidden_states)
        .reshape(BT, HB, P)
        .transpose(2, 1, 0)
    ).astype(bf)
    wq_b, wk_b, wv_b, wo_b = (np.asarray(w).astype(bf) for w in (wq, wk, wv, wo))

    def flat128(w):
        # [nb*128, D] -> [128, nb*D]: row (c*128+p) col j -> [p, c*D+j]
        nb, d = w.shape[0] // P, w.shape[1]
        return np.ascontiguousarray(
            w.reshape(nb, P, d).transpose(1, 0, 2).reshape(P, nb * d)
        )

    # rope tables in transposed layout [128 d, BT t]
    half = P // 2
    inv_freq = 1.0 / (10000.0 ** (np.arange(half, dtype=np.float64) / half))
    pos = np.asarray(position_ids).astype(np.float64).reshape(BT)
    ang = pos[None, :] * inv_freq[:, None]  # [64, BT]
    cos_t = np.concatenate([np.cos(ang), np.cos(ang)], 0).astype(np.float32)
    sinx_t = np.concatenate([-np.sin(ang), np.sin(ang)], 0).astype(np.float32)

    # consts: [ones for L row-sums, triangular causal mask tri[k,q]=k<=q]
    tri = np.arange(P)[:, None] <= np.arange(P)[None, :]
    consts_t = np.stack([np.ones((P, P)), tri]).astype(bf)

    in_maps = []
    for c in range(NCORES):
        in_maps.append(
            {
                "xr": xr,
                "wq_r": flat128(wq_b[:, c * DQ : (c + 1) * DQ]),
                "wk_r": flat128(wk_b[:, c * P : (c + 1) * P]),
                "wv_r": flat128(wv_b[:, c * P : (c + 1) * P]),
                "wo_rr": flat128(wo_b[c * DQ : (c + 1) * DQ, :]),
                "cos_t": cos_t,
                "sinx_t": sinx_t,
                "consts_t": consts_t,
            }
        )
    return in_maps


def run(hidden_states, wq, wk, wv, wo, position_ids, B, S, H, NH, NKV, trace=False):
    key = (B, S, H, NH, NKV)
    if key not in _CACHE:
        _CACHE[key] = build_program(B, S, H, NH, NKV)
    nc = _CACHE[key]
    in_maps = _prep_inputs(
        hidden_states, wq, wk, wv, wo, position_ids, B, S, H, NH, NKV
    )
    res = run_bass_kernel_spmd(nc, in_maps, core_ids=list(range(NCORES)), trace=trace)
    acc = np.zeros((B * S, H), np.float32)
    for c in range(NCORES):
        acc += np.asarray(res.results[c]["y_c"], dtype=np.float32)
    out = acc.reshape(B, S, H)
    return (out, res) if trace else (out, None)


def kernel(hidden_states, wq, wk, wv, wo, position_ids):
    out, _ = run(
        hidden_states, wq, wk, wv, wo, position_ids, 2, 2048, 4096, 32, 8
    )
    return out
